# revision 1
# baseline (speedup 1.0000x reference)
"""LLaMA layer (B=2, T=1024, D=2048, H=16 GQA-4, F=5632) on 8 trn2 NeuronCores.

Sharding: heads/FFN tensor-parallel + sequence-parallel norms.
  - core c owns q-heads {2c, 2c+1}, kv-head c//2, FFN cols [c*704, (c+1)*704),
    and token chunk [c*256, (c+1)*256) of the 2048 global (b-major) tokens.
  - collectives: AllGather(xn), AllToAll(attn out: heads -> token chunks),
    AllGather(h). FFN partial sums are reduced on the host.
All matmuls run in float32r (full PE rate), accumulating in fp32 PSUM.
"""

import numpy as np

NC = 8
B, T, D = 2, 1024, 2048
H, HKV, DH = 16, 4, 128
F = 5632
FC = F // NC            # 704
GLOB = B * T            # 2048 tokens, b-major
TOK = GLOB // NC        # 256 own tokens
DT = D // 128           # 16 D-tiles
NG = GLOB // 512        # 4 moving groups of 512 tokens
EPS = 1e-6
SCL = DH ** -0.5
ROPE_THETA = 10000.0

_CACHE = {}


class _Stop(Exception):
    pass


def _build_program(upto=8):
    import concourse.bass as bass
    import concourse.mybir as mybir
    import concourse.tile as tile
    from concourse import bacc
    from concourse.masks import make_identity

    F32 = mybir.dt.float32
    F32R = mybir.dt.float32r
    AF = mybir.ActivationFunctionType

    nc = bacc.Bacc("TRN2", target_bir_lowering=False, debug=False,
                   enable_asserts=False, num_devices=NC)

    # ---- per-core inputs (host pre-sliced / pre-folded) ----
    xc = nc.dram_tensor("xc", [TOK, D], F32, kind="ExternalInput").ap()
    xf = nc.dram_tensor("xf", [GLOB, D], F32, kind="ExternalInput").ap()
    wq = nc.dram_tensor("wq", [D, 2 * DH], F32R, kind="ExternalInput").ap()
    wk = nc.dram_tensor("wk", [D, DH], F32R, kind="ExternalInput").ap()
    wv = nc.dram_tensor("wv", [D, DH], F32R, kind="ExternalInput").ap()
    wo = nc.dram_tensor("wo", [DT, 128, DT * 128], F32R, kind="ExternalInput").ap()
    w1 = nc.dram_tensor("w1", [D, FC], F32R, kind="ExternalInput").ap()
    w3 = nc.dram_tensor("w3", [D, FC], F32R, kind="ExternalInput").ap()
    w2 = nc.dram_tensor("w2", [FC, D], F32R, kind="ExternalInput").ap()
    cscat = nc.dram_tensor("cscat", [128, GLOB], F32, kind="ExternalInput").ap()
    sccat = nc.dram_tensor("sccat", [128, GLOB], F32, kind="ExternalInput").ap()
    masks = nc.dram_tensor("masks", [4, 128, 512], F32, kind="ExternalInput").ap()

    # ---- per-core outputs ----
    x1t = nc.dram_tensor("x1t", [D, TOK], F32, kind="ExternalOutput").ap()
    ffnt = nc.dram_tensor("ffnt", [D, GLOB], F32, kind="ExternalOutput").ap()

    RG = [list(range(NC))]

    with tile.TileContext(nc) as tc:
      try:
        with tc.tile_pool(name="const", bufs=1) as cp, \
             tc.tile_pool(name="dram", bufs=1, space="DRAM") as dp:
            # constants
            ident = cp.tile([128, 128], F32, name="ident")
            make_identity(nc, ident[:])
            identr = cp.tile([128, 128], F32R, name="identr")
            nc.vector.tensor_copy(identr[:], ident[:])
            ones_c32 = cp.tile([128, 1], F32, name="ones_c32")
            nc.vector.memset(ones_c32[:], 1.0)
            ones_c = cp.tile([128, 1], F32R, name="ones_c")
            nc.vector.tensor_copy(ones_c[:], ones_c32[:])
            ones_r32 = cp.tile([1, 128], F32, name="ones_r32")
            nc.vector.memset(ones_r32[:], 1.0)
            ones_r = cp.tile([1, 128], F32R, name="ones_r")
            nc.vector.tensor_copy(ones_r[:], ones_r32[:])
            eps128 = cp.tile([128, 1], F32, name="eps128")
            nc.vector.memset(eps128[:], EPS)
            scd128 = cp.tile([128, 1], F32, name="scd128")
            nc.vector.memset(scd128[:], 1.0 / D)
            eps1 = cp.tile([1, 1], F32, name="eps1")
            nc.vector.memset(eps1[:], EPS)
            scd1 = cp.tile([1, 1], F32, name="scd1")
            nc.vector.memset(scd1[:], 1.0 / D)
            scexp = cp.tile([128, 1], F32, name="scexp")
            nc.vector.memset(scexp[:], SCL)

            # DRAM bounce buffers for collectives
            o_in = dp.tile([NC, 2 * DH, TOK], F32R, name="o_in")
            o_out = dp.tile([NC, 2 * DH, TOK], F32R, name="o_out")
            h_in = dp.tile([D, TOK], F32R, name="h_in")
            h_all = dp.tile([NC * D, TOK], F32R, name="h_all", addr_space="Shared")

            with tc.tile_pool(name="resid", bufs=1) as rp:
                xT = [rp.tile([128, TOK], F32, name=f"xT{d}") for d in range(DT)]

                with tc.tile_pool(name="tabs", bufs=1) as tb, \
                     tc.tile_pool(name="acts", bufs=1) as ac, \
                     tc.tile_pool(name="oTp", bufs=1) as op_:
                    cs_cat = tb.tile([128, GLOB], F32, name="cs_cat")
                    sc_cat = tb.tile([128, GLOB], F32, name="sc_cat")
                    nc.sync.dma_start(cs_cat[:], cscat[:])
                    nc.sync.dma_start(sc_cat[:], sccat[:])

                    # ======== phase B: fused norm1 + QKV + RoPE ========
                    if upto < 3:
                        raise _Stop()
                    qT = [ac.tile([128, GLOB], F32R, name=f"qT{h}") for h in range(2)]
                    kT = ac.tile([128, GLOB], F32R, name="kT")
                    Vn4 = [ac.tile([128, 512], F32R, name=f"Vn4{t}") for t in range(4)]

                    with tc.tile_pool(name="wqkv", bufs=1) as wp, \
                         tc.tile_pool(name="phB", bufs=1) as pb, \
                         tc.tile_pool(name="psB", bufs=1, space="PSUM") as psB:
                        wq_r = wq.rearrange("(a p) m -> a p m", p=128)
                        wk_r = wk.rearrange("(a p) m -> a p m", p=128)
                        wv_r = wv.rearrange("(a p) m -> a p m", p=128)
                        wq_sb, wk_sb, wv_sb = [], [], []
                        for k in range(DT):
                            tq_ = wp.tile([128, 2 * DH], F32R, name=f"wq_sb{k}")
                            nc.sync.dma_start(tq_[:], wq_r[k])
                            wq_sb.append(tq_)
                            tk_ = wp.tile([128, DH], F32R, name=f"wk_sb{k}")
                            nc.sync.dma_start(tk_[:], wk_r[k])
                            wk_sb.append(tk_)
                            tv_ = wp.tile([128, DH], F32R, name=f"wv_sb{k}")
                            nc.sync.dma_start(tv_[:], wv_r[k])
                            wv_sb.append(tv_)
                        for g in range(NG):
                            gc = slice(g * 512, (g + 1) * 512)
                            # raw x^T tiles (norm scale is applied post-projection)
                            rhs = [pb.tile([128, 512], F32R, name=f"xnT{d}_{g}",
                                           tag=f"xnT{d}", bufs=1) for d in range(DT)]
                            xg = []
                            invp = psB.tile([1, 512], F32, name=f"invp{g}",
                                            tag="invp", bufs=1)
                            for tt in range(4):
                                x_ = pb.tile([128, D], F32, name=f"xg{g}_{tt}",
                                             tag=f"xg{tt}", bufs=1)
                                t0 = g * 512 + tt * 128
                                nc.sync.dma_start(x_[:], xf[t0:t0 + 128, :])
                                xg.append(x_)
                                # norm stats (ACT; two half-width squares)
                                scr = pb.tile([128, D // 2], F32,
                                              name=f"sqscr{g}_{tt}",
                                              tag="sqscr", bufs=1)
                                sa = pb.tile([128, 1], F32, name=f"ssqa{g}_{tt}",
                                             tag="ssqa", bufs=2)
                                sb_ = pb.tile([128, 1], F32, name=f"ssqb{g}_{tt}",
                                              tag="ssqb", bufs=2)
                                nc.scalar.activation(scr[:], x_[:, 0:D // 2],
                                                     AF.Square, accum_out=sa[:])
                                nc.scalar.activation(scr[:], x_[:, D // 2:D],
                                                     AF.Square, accum_out=sb_[:])
                                ssq = pb.tile([128, 1], F32, name=f"ssq{g}_{tt}",
                                              tag="ssq", bufs=2)
                                nc.vector.tensor_add(ssq[:], sa[:], sb_[:])
                                std = pb.tile([128, 1], F32, name=f"std{g}_{tt}",
                                              tag="std", bufs=2)
                                nc.scalar.activation(std[:], ssq[:], AF.Sqrt,
                                                     scale=scd128[:], bias=eps128[:])
                                inv = pb.tile([128, 1], F32, name=f"inv{g}_{tt}",
                                              tag="inv", bufs=2)
                                nc.vector.reciprocal(inv[:], std[:])
                                nc.tensor.transpose(
                                    invp[0:1, tt * 128:(tt + 1) * 128], inv[:],
                                    ident[:])
                            invrow = pb.tile([1, 512], F32R, name=f"invrow{g}",
                                             tag="invrow", bufs=2)
                            nc.vector.tensor_copy(invrow[:], invp[:])
                            ibp = psB.tile([128, 512], F32, name=f"ibp{g}",
                                           tag="ibp", bufs=1)
                            nc.tensor.matmul(ibp[:], ones_r[:], invrow[:],
                                             start=True, stop=True)
                            ibs = pb.tile([128, 512], F32, name=f"ibs{g}",
                                          tag="ibs", bufs=2)
                            nc.vector.tensor_copy(ibs[:], ibp[:])
                            # rope tables pre-scaled by inv (fold norm into rope)
                            csx = pb.tile([128, 512], F32, name=f"csx{g}",
                                          tag="csx", bufs=1)
                            scx = pb.tile([128, 512], F32, name=f"scx{g}",
                                          tag="scx", bufs=1)
                            nc.vector.tensor_mul(csx[:], cs_cat[:, gc], ibs[:])
                            nc.vector.tensor_mul(scx[:], sc_cat[:, gc], ibs[:])
                            # transposes, batched 4 -> one psum bank, one copy
                            for d in range(DT):
                                tpD = psB.tile([128, 512], F32,
                                               name=f"tpD{g}_{d}",
                                               tag="tpD", bufs=3)
                                for tt in range(4):
                                    nc.tensor.transpose(
                                        tpD[:, tt * 128:(tt + 1) * 128],
                                        xg[tt][:, d * 128:(d + 1) * 128], ident[:])
                                if d % 2 == 0:
                                    nc.vector.tensor_copy(rhs[d][:], tpD[:])
                                else:
                                    nc.scalar.copy(rhs[d][:], tpD[:])

                            def rope(ps, dst, tag):
                                a = pb.tile([64, 512], F32, name=f"ra_{tag}",
                                            tag="ra", bufs=1)
                                b_ = pb.tile([64, 512], F32, name=f"rb_{tag}",
                                             tag="rb", bufs=1)
                                nc.vector.tensor_mul(a[:], ps[0:64, :], csx[0:64, :])
                                nc.vector.tensor_mul(b_[:], ps[64:128, :],
                                                     csx[64:128, :])
                                nc.vector.tensor_sub(dst[0:64, gc], a[:], b_[:])
                                c_ = pb.tile([64, 512], F32, name=f"rc_{tag}",
                                             tag="rc", bufs=1)
                                d_ = pb.tile([64, 512], F32, name=f"rd_{tag}",
                                             tag="rd", bufs=1)
                                nc.vector.tensor_mul(c_[:], ps[0:64, :], scx[0:64, :])
                                nc.vector.tensor_mul(d_[:], ps[64:128, :],
                                                     scx[64:128, :])
                                nc.vector.tensor_add(dst[64:128, gc], c_[:], d_[:])

                            for hl in range(2):
                                ps = psB.tile([128, 512], F32, name=f"psq{hl}_{g}",
                                              tag="pqkv", bufs=2)
                                for k in range(DT):
                                    nc.tensor.matmul(
                                        ps[:], wq_sb[k][:, hl * DH:(hl + 1) * DH],
                                        rhs[k][:], start=(k == 0), stop=(k == DT - 1))
                                rope(ps, qT[hl], f"q{hl}_{g}")
                            ps = psB.tile([128, 512], F32, name=f"psk_{g}",
                                          tag="pqkv", bufs=2)
                            for k in range(DT):
                                nc.tensor.matmul(ps[:], wk_sb[k][:], rhs[k][:],
                                                 start=(k == 0), stop=(k == DT - 1))
                            rope(ps, kT, f"k{g}")
                            ps = psB.tile([128, 512], F32, name=f"psv_{g}",
                                          tag="pqkv", bufs=2)
                            for k in range(DT):
                                nc.tensor.matmul(ps[:], wv_sb[k][:], rhs[k][:],
                                                 start=(k == 0), stop=(k == DT - 1))
                            vTg = pb.tile([128, 512], F32R, name=f"vTg{g}",
                                          tag="vTg", bufs=1)
                            nc.vector.tensor_mul(vTg[:], ps[:], ibs[:])
                            tpV = psB.tile([128, 512], F32R, name=f"tpV{g}",
                                           tag="tpV", bufs=1)
                            for tt in range(4):
                                nc.tensor.transpose(
                                    tpV[:, tt * 128:(tt + 1) * 128],
                                    vTg[:, tt * 128:(tt + 1) * 128], identr[:])
                            nc.scalar.copy(Vn4[g][:], tpV[:])

                    # ======== phase C: attention ========
                    if upto < 4:
                        raise _Stop()
                    oT = [op_.tile([128, GLOB], F32R, name=f"oT{h}") for h in range(2)]
                    with tc.tile_pool(name="phC", bufs=1) as pc, \
                         tc.tile_pool(name="psC", bufs=1, space="PSUM") as psC:
                        msk = [pc.tile([128, 512], F32, name=f"msk{v}")
                               for v in range(4)]
                        for v in range(4):
                            nc.sync.dma_start(msk[v][:], masks[v])
                        for b2 in range(B):
                            for hl in range(2):
                                for qg in range(2):
                                    qc = slice(b2 * T + qg * 512, b2 * T + (qg + 1) * 512)
                                    nkt = 4 * (qg + 1)
                                    pso = psC.tile([128, 512], F32,
                                                   name=f"pso{b2}{hl}{qg}",
                                                   tag="pso", bufs=2)
                                    pssum = psC.tile([1, 512], F32,
                                                     name=f"pssum{b2}{hl}{qg}",
                                                     tag="pssum", bufs=2)
                                    for kt in range(nkt):
                                        pss = psC.tile([128, 512], F32,
                                                       name=f"pss{b2}{hl}{qg}{kt}",
                                                       tag="pss", bufs=3)
                                        k0 = b2 * T + kt * 128
                                        nc.tensor.matmul(
                                            pss[:], kT[:, k0:k0 + 128], qT[hl][:, qc],
                                            start=True, stop=True)
                                        e = pc.tile([128, 512], F32R,
                                                    name=f"e{b2}{hl}{qg}{kt}",
                                                    tag="e", bufs=4)
                                        nc.scalar.activation(e[:], pss[:], AF.Exp,
                                                             scale=scexp[:])
                                        v = kt - 4 * qg
                                        if 0 <= v <= 3:
                                            em = pc.tile([128, 512], F32R,
                                                         name=f"em{b2}{hl}{qg}{kt}",
                                                         tag="em", bufs=2)
                                            nc.vector.tensor_mul(em[:], e[:], msk[v][:])
                                            eu = em
                                        else:
                                            eu = e
                                        nc.tensor.matmul(
                                            pssum[:], ones_c[:], eu[:],
                                            start=(kt == 0), stop=(kt == nkt - 1))
                                        gt = b2 * 8 + kt
                                        nc.tensor.matmul(
                                            pso[:],
                                            Vn4[gt // 4][:, (gt % 4) * 128:
                                                         (gt % 4 + 1) * 128],
                                            eu[:],
                                            start=(kt == 0), stop=(kt == nkt - 1))
                                    rec = pc.tile([1, 512], F32R,
                                                  name=f"rec{b2}{hl}{qg}",
                                                  tag="rec", bufs=2)
                                    with nc.allow_low_precision(
                                            reason="f32r softmax recip"):
                                        nc.vector.reciprocal(rec[:], pssum[:])
                                    rbc = psC.tile([128, 512], F32,
                                                   name=f"rbc{b2}{hl}{qg}",
                                                   tag="rbc", bufs=1)
                                    nc.tensor.matmul(rbc[:], ones_r[:], rec[:],
                                                     start=True, stop=True)
                                    rbs = pc.tile([128, 512], F32,
                                                  name=f"rbs{b2}{hl}{qg}",
                                                  tag="rbs", bufs=2)
                                    nc.vector.tensor_copy(rbs[:], rbc[:])
                                    nc.vector.tensor_mul(oT[hl][:, qc], pso[:], rbs[:])
                    # ======== phase A: residual x^T tiles for own chunk ========
                    with tc.tile_pool(name="phA", bufs=1) as pa, \
                         tc.tile_pool(name="psA", bufs=1, space="PSUM") as psA:
                        xts = []
                        for i in range(2):
                            t = pa.tile([128, D], F32, name=f"xts{i}")
                            nc.sync.dma_start(t[:], xc[i * 128:(i + 1) * 128, :])
                            xts.append(t)
                        for d in range(DT):
                            for i in range(2):
                                tp = psA.tile([128, 128], F32, name=f"tpx{d}_{i}",
                                              tag="tpx", bufs=4)
                                nc.tensor.transpose(
                                    tp[:], xts[i][:, d * 128:(d + 1) * 128], ident[:])
                                nc.vector.tensor_copy(
                                    xT[d][:, i * 128:(i + 1) * 128], tp[:])

                    # A2A: my heads x all tokens -> all heads x my tokens
                    for hl in range(2):
                        for j in range(NC):
                            nc.sync.dma_start(
                                o_in[j, hl * 128:(hl + 1) * 128, :],
                                oT[hl][:, j * TOK:(j + 1) * TOK])
                if upto < 5:
                    raise _Stop()
                nc.gpsimd.collective_compute(
                    "AllToAll", mybir.AluOpType.bypass, replica_groups=RG,
                    ins=[o_in[:]], outs=[o_out[:]])

                # ======== phase D: o-proj + residual + norm2 + AG(h) ========
                if upto < 6:
                    raise _Stop()
                with tc.tile_pool(name="phD", bufs=1) as pd, \
                     tc.tile_pool(name="psD", bufs=1, space="PSUM") as psD:
                    oT_own = []
                    oo = o_out.rearrange("r (a p) t -> (r a) p t", p=128)
                    for k in range(DT):
                        t_ = pd.tile([128, TOK], F32R, name=f"oT_own{k}")
                        nc.sync.dma_start(t_[:], oo[k])
                        oT_own.append(t_)
                    x1T = []
                    for d in range(DT):
                        wos = pd.tile([128, DT * 128], F32R, name=f"wos{d}",
                                      tag="wos", bufs=10)
                        nc.sync.dma_start(wos[:], wo[d])
                        pso2 = psD.tile([128, TOK], F32, name=f"pso2_{d}",
                                        tag="pso2", bufs=2)
                        for k in range(DT):
                            nc.tensor.matmul(
                                pso2[:], wos[:, k * 128:(k + 1) * 128],
                                oT_own[k][:], start=(k == 0), stop=(k == DT - 1))
                        xt_ = pd.tile([128, TOK], F32, name=f"x1T{d}")
                        nc.vector.tensor_add(xt_[:], pso2[:], xT[d][:])
                        nc.sync.dma_start(x1t[d * 128:(d + 1) * 128, :], xt_[:])
                        x1T.append(xt_)
                    # norm2 (transposed): ssq over partitions via ones-matmul
                    ssq2 = psD.tile([1, TOK], F32, name="ssq2")
                    for d in range(DT):
                        sq2 = pd.tile([128, TOK], F32R, name=f"sq2_{d}",
                                      tag="sq2", bufs=2)
                        nc.scalar.activation(sq2[:], x1T[d][:], AF.Square)
                        nc.tensor.matmul(ssq2[:], ones_c[:], sq2[:],
                                         start=(d == 0), stop=(d == DT - 1))
                    std2 = pd.tile([1, TOK], F32, name="std2")
                    nc.scalar.activation(std2[:], ssq2[:], AF.Sqrt,
                                         scale=scd1[:], bias=eps1[:])
                    inv2 = pd.tile([1, TOK], F32R, name="inv2")
                    with nc.allow_low_precision(reason="f32r norm2 recip"):
                        nc.vector.reciprocal(inv2[:], std2[:])
                    i2p = psD.tile([128, TOK], F32, name="i2p")
                    nc.tensor.matmul(i2p[:], ones_r[:], inv2[:], start=True, stop=True)
                    i2s = pd.tile([128, TOK], F32, name="i2s")
                    nc.vector.tensor_copy(i2s[:], i2p[:])
                    for d in range(DT):
                        hT = pd.tile([128, TOK], F32R, name=f"hT{d}",
                                     tag="hT", bufs=3)
                        nc.vector.tensor_mul(hT[:], x1T[d][:], i2s[:])
                        nc.sync.dma_start(h_in[d * 128:(d + 1) * 128, :], hT[:])
            if upto < 7:
                raise _Stop()
            nc.gpsimd.collective_compute(
                "AllGather", mybir.AluOpType.bypass, replica_groups=RG,
                ins=[h_in[:]], outs=[h_all[:]])

            # ======== phase E: FFN (F-sharded partial) ========
            if upto < 8:
                raise _Stop()
            FT = [128, 128, 128, 128, 128, 64]  # 704 = 5*128 + 64
            with tc.tile_pool(name="wf", bufs=1) as wf, \
                 tc.tile_pool(name="phE", bufs=1) as pe, \
                 tc.tile_pool(name="psE", bufs=1, space="PSUM") as psE:
                w1_r = w1.rearrange("(a p) m -> a p m", p=128)
                w3_r = w3.rearrange("(a p) m -> a p m", p=128)
                w1_sb, w3_sb, w2_sb = [], [], []
                for k in range(DT):
                    t1 = wf.tile([128, FC], F32R, name=f"w1_sb{k}")
                    nc.sync.dma_start(t1[:], w1_r[k])
                    w1_sb.append(t1)
                    t3 = wf.tile([128, FC], F32R, name=f"w3_sb{k}")
                    nc.sync.dma_start(t3[:], w3_r[k])
                    w3_sb.append(t3)
                for ft in range(6):
                    f0 = ft * 128
                    t2 = wf.tile([FT[ft], D], F32R, name=f"w2_sb{ft}")
                    nc.sync.dma_start(t2[:], w2[f0:f0 + FT[ft], :])
                    w2_sb.append(t2)
                ha = h_all.rearrange("(r a p) t -> r a p t", r=NC, p=128)
                for g in range(NG):
                    gc = slice(g * 512, (g + 1) * 512)
                    rhs = []
                    for d in range(DT):
                        r_ = pe.tile([128, 512], F32R, name=f"hr{d}_{g}",
                                     tag=f"hr{d}", bufs=1)
                        nc.sync.dma_start(r_[:, 0:TOK], ha[2 * g, d])
                        nc.sync.dma_start(r_[:, TOK:512], ha[2 * g + 1, d])
                        rhs.append(r_)
                    zT = []
                    for ft in range(6):
                        fp_ = FT[ft]
                        f0 = ft * 128
                        pg = psE.tile([fp_, 512], F32, name=f"pg{ft}_{g}",
                                      tag="pg", bufs=2)
                        for k in range(DT):
                            nc.tensor.matmul(pg[:], w1_sb[k][:, f0:f0 + fp_],
                                             rhs[k][:], start=(k == 0),
                                             stop=(k == DT - 1))
                        pu = psE.tile([fp_, 512], F32, name=f"pu{ft}_{g}",
                                      tag="pu", bufs=2)
                        for k in range(DT):
                            nc.tensor.matmul(pu[:], w3_sb[k][:, f0:f0 + fp_],
                                             rhs[k][:], start=(k == 0),
                                             stop=(k == DT - 1))
                        sil = pe.tile([fp_, 512], F32, name=f"sil{ft}_{g}",
                                      tag="sil", bufs=3)
                        nc.scalar.activation(sil[:], pg[:], AF.Silu)
                        z_ = pe.tile([fp_, 512], F32R, name=f"zT{ft}_{g}",
                                     tag=f"zT{ft}", bufs=2)
                        nc.vector.tensor_mul(z_[:], sil[:], pu[:])
                        zT.append(z_)
                    for d in range(DT):
                        pf = psE.tile([128, 512], F32, name=f"pf{d}_{g}",
                                      tag="pf", bufs=3)
                        for ft in range(6):
                            nc.tensor.matmul(pf[:], w2_sb[ft][:, d * 128:(d + 1) * 128],
                                             zT[ft][:], start=(ft == 0),
                                             stop=(ft == 5))
                        fo = pe.tile([128, 512], F32, name=f"fo{d}_{g}",
                                     tag="fo", bufs=3)
                        nc.vector.tensor_copy(fo[:], pf[:])
                        nc.sync.dma_start(ffnt[d * 128:(d + 1) * 128, gc], fo[:])
      except _Stop:
        pass
    nc.compile()
    return nc


def _prep_inputs(inputs):
    x = np.asarray(inputs["x"], np.float32)
    cos = np.asarray(inputs["freqs_cos"], np.float32)
    sin = np.asarray(inputs["freqs_sin"], np.float32)
    wn1 = np.asarray(inputs["w_norm1"], np.float32)[:, None]
    wn2 = np.asarray(inputs["w_norm2"], np.float32)[:, None]
    wq = np.asarray(inputs["wq"], np.float32) * wn1
    wk = np.asarray(inputs["wk"], np.float32) * wn1
    wv = np.asarray(inputs["wv"], np.float32) * wn1
    wo = np.asarray(inputs["wo"], np.float32)
    w1 = np.asarray(inputs["w1"], np.float32) * wn2
    w3 = np.asarray(inputs["w3"], np.float32) * wn2
    w2 = np.asarray(inputs["w2"], np.float32)

    xg = np.ascontiguousarray(x.reshape(GLOB, D))
    perm = np.concatenate([np.arange(0, DH, 2), np.arange(1, DH, 2)])
    cosT = np.concatenate([cos.T, cos.T], axis=1)
    sinT = np.concatenate([sin.T, sin.T], axis=1)
    cscat = np.ascontiguousarray(np.concatenate([cosT, sinT], axis=0))
    sccat = np.ascontiguousarray(np.concatenate([sinT, cosT], axis=0))
    mk = np.zeros((4, 128, 512), np.float32)
    for v in range(4):
        r = np.arange(128)[:, None] + v * 128
        q = np.arange(512)[None, :]
        mk[v] = (r <= q).astype(np.float32)

    wo_sw = np.ascontiguousarray(
        wo.reshape(DT, 128, DT, 128).transpose(2, 1, 0, 3).reshape(DT, 128, DT * 128))
    in_maps = []
    for c in range(NC):
        g = c // 2
        wq_c = np.empty((D, 2 * DH), np.float32)
        for hl in range(2):
            h = 2 * c + hl
            wq_c[:, hl * DH:(hl + 1) * DH] = wq[:, h * DH + perm]
        wk_c = wk[:, g * DH + perm]
        wv_c = wv[:, g * DH:(g + 1) * DH]
        in_maps.append({
            "xc": np.ascontiguousarray(xg[c * TOK:(c + 1) * TOK, :]),
            "xf": xg,
            "wq": np.ascontiguousarray(wq_c),
            "wk": np.ascontiguousarray(wk_c),
            "wv": np.ascontiguousarray(wv_c),
            "wo": wo_sw,
            "w1": np.ascontiguousarray(w1[:, c * FC:(c + 1) * FC]),
            "w3": np.ascontiguousarray(w3[:, c * FC:(c + 1) * FC]),
            "w2": np.ascontiguousarray(w2[c * FC:(c + 1) * FC, :]),
            "cscat": cscat,
            "sccat": sccat,
            "masks": mk,
        })
    return in_maps


def kernel(**inputs) -> np.ndarray:
    from concourse import bass_utils

    if "nc" not in _CACHE:
        _CACHE["nc"] = _build_program()
    nc = _CACHE["nc"]
    in_maps = _prep_inputs(inputs)
    res = bass_utils.run_bass_kernel_spmd(nc, in_maps, core_ids=list(range(NC)))
    yT = np.zeros((D, GLOB), np.float64)
    for c in range(NC):
        yT += res.results[c]["ffnt"].astype(np.float64)
    for c in range(NC):
        yT[:, c * TOK:(c + 1) * TOK] += res.results[c]["x1t"].astype(np.float64)
    return np.ascontiguousarray(yT.T).astype(np.float32).reshape(B, T, D)


if __name__ == "__main__":
    import reference
    inputs = {k: np.asarray(v) for k, v in reference.setup_inputs().items()}
    out = kernel(**inputs)
    print("kernel output shape:", out.shape)



# revision 4
# speedup vs baseline: 1.8001x; 1.8001x over previous
"""LLaMA layer (B=2, T=1024, D=2048, H=16 GQA-4, F=5632) on 8 trn2 NeuronCores.

v2 sharding: heads tensor-parallel for attention + token-parallel FFN.
  - core c owns q-heads {2c, 2c+1}, kv-head c//2 for attention over ALL
    tokens, and token chunk [c*256, (c+1)*256) of the 2048 global (b-major)
    tokens for o-proj/residual/norm2/FFN.
  - single collective: AllToAll (bf16) of attention outputs
    (my heads x all tokens -> all heads x my tokens).
  - FFN is token-sharded: full w1/w3/w2 streamed to every core in bf16
    (DMA overlaps PE), no AllGather and no partial-sum reduction.
  - norm1 is folded on the host (x pre-normalized + pre-transposed);
    norm weights are folded into the projection weights.
All matmuls run in bf16 (full PE rate), accumulating in fp32 PSUM.
"""

import numpy as np

NC = 8
B, T, D = 2, 1024, 2048
H, HKV, DH = 16, 4, 128
F = 5632
GLOB = B * T            # 2048 tokens, b-major
TOK = GLOB // NC        # 256 own tokens
DT = D // 128           # 16 D-tiles
NG = GLOB // 512        # 4 groups of 512 tokens
NFT = F // 128          # 44 FFN f-tiles
NFP = NFT // 2          # 22 ft-pairs (w1/w3 stream granularity)
EPS = 1e-6
SCL = DH ** -0.5

_CACHE = {}


def _build_program():
    import concourse.bass as bass
    import concourse.mybir as mybir
    import concourse.tile as tile
    from concourse import bacc

    F32 = mybir.dt.float32
    F32R = mybir.dt.float32r
    BF16 = mybir.dt.bfloat16
    AF = mybir.ActivationFunctionType

    nc = bacc.Bacc("TRN2", target_bir_lowering=False, debug=False,
                   enable_asserts=False, num_devices=NC)

    # ---- per-core inputs (host pre-sliced / pre-folded) ----
    xnt = nc.dram_tensor("xnt", [DT, 128, GLOB], BF16, kind="ExternalInput").ap()
    xtc = nc.dram_tensor("xtc", [DT, 128, TOK], F32, kind="ExternalInput").ap()
    wq = nc.dram_tensor("wq", [DT, 128, 2 * DH], BF16, kind="ExternalInput").ap()
    wk = nc.dram_tensor("wk", [DT, 128, DH], BF16, kind="ExternalInput").ap()
    wv = nc.dram_tensor("wv", [DT, 128, DH], BF16, kind="ExternalInput").ap()
    wo = nc.dram_tensor("wo", [DT, 128, D], BF16, kind="ExternalInput").ap()
    w1p = nc.dram_tensor("w1p", [NFP, 128, 4096], BF16, kind="ExternalInput").ap()
    w3p = nc.dram_tensor("w3p", [NFP, 128, 4096], BF16, kind="ExternalInput").ap()
    w2d = nc.dram_tensor("w2d", [DT, 128, F], BF16, kind="ExternalInput").ap()
    cscat = nc.dram_tensor("cscat", [128, GLOB], BF16, kind="ExternalInput").ap()
    sccat = nc.dram_tensor("sccat", [128, GLOB], BF16, kind="ExternalInput").ap()
    masks = nc.dram_tensor("masks", [4, 128, 512], BF16, kind="ExternalInput").ap()

    # ---- per-core output: x1 + ffn for own tokens, transposed ----
    yt = nc.dram_tensor("yt", [D, TOK], F32, kind="ExternalOutput").ap()

    RG = [list(range(NC))]

    with tile.TileContext(nc) as tc:
        with tc.tile_pool(name="const", bufs=1) as cp, \
             tc.tile_pool(name="dram", bufs=1, space="DRAM") as dp:
            # constants
            ones_c32 = cp.tile([128, 1], F32, name="ones_c32")
            nc.vector.memset(ones_c32[:], 1.0)
            ones_cb = cp.tile([128, 1], BF16, name="ones_cb")
            nc.vector.tensor_copy(ones_cb[:], ones_c32[:])
            ones_cr = cp.tile([128, 1], F32R, name="ones_cr")
            nc.vector.tensor_copy(ones_cr[:], ones_c32[:])
            ones_r32 = cp.tile([1, 128], F32, name="ones_r32")
            nc.vector.memset(ones_r32[:], 1.0)
            ones_r = cp.tile([1, 128], F32R, name="ones_r")
            nc.vector.tensor_copy(ones_r[:], ones_r32[:])
            eps1 = cp.tile([1, 1], F32, name="eps1")
            nc.vector.memset(eps1[:], EPS)
            scd1 = cp.tile([1, 1], F32, name="scd1")
            nc.vector.memset(scd1[:], 1.0 / D)
            scexp = cp.tile([128, 1], F32, name="scexp")
            nc.vector.memset(scexp[:], SCL)

            # DRAM bounce buffers for the A2A collective
            o_in = dp.tile([NC, 2 * DH, TOK], BF16, name="o_in")
            o_out = dp.tile([NC, 2 * DH, TOK], BF16, name="o_out")

            with tc.tile_pool(name="resid", bufs=1) as rp:
                # persistent activations
                qT = [rp.tile([128, GLOB], BF16, name=f"qT{h}") for h in range(2)]
                kT = rp.tile([128, GLOB], BF16, name="kT")
                Vn = [rp.tile([128, DH], BF16, name=f"Vn{t}") for t in range(16)]
                oT = [rp.tile([128, GLOB], BF16, name=f"oT{h}") for h in range(2)]
                xts = rp.tile([128, DT * TOK], F32, name="xts")
                x1T = [rp.tile([128, TOK], F32, name=f"x1T{d}") for d in range(DT)]
                hT = [rp.tile([128, TOK], BF16, name=f"hT{d}") for d in range(DT)]
                zT = [rp.tile([128, TOK], BF16, name=f"zT{ft}") for ft in range(NFT)]

                # raw x^T for the residual (one DMA)
                nc.sync.dma_start(
                    xts[:].rearrange("p (a t) -> a p t", a=DT), xtc[:])

                with tc.tile_pool(name="tabs", bufs=1) as tb:
                    cs_cat = tb.tile([128, GLOB], BF16, name="cs_cat")
                    sc_cat = tb.tile([128, GLOB], BF16, name="sc_cat")
                    nc.sync.dma_start(cs_cat[:], cscat[:])
                    nc.sync.dma_start(sc_cat[:], sccat[:])
                    msk = tb.tile([128, 4 * 512], BF16, name="msk")
                    nc.sync.dma_start(
                        msk[:].rearrange("p (v t) -> v p t", v=4), masks[:])
                    wq_sb = tb.tile([128, DT * 2 * DH], BF16, name="wq_sb")
                    nc.scalar.dma_start(
                        wq_sb[:].rearrange("p (a m) -> a p m", a=DT), wq[:])
                    wk_sb = tb.tile([128, DT * DH], BF16, name="wk_sb")
                    nc.scalar.dma_start(
                        wk_sb[:].rearrange("p (a m) -> a p m", a=DT), wk[:])
                    wv_sb = tb.tile([128, DT * DH], BF16, name="wv_sb")
                    nc.scalar.dma_start(
                        wv_sb[:].rearrange("p (a m) -> a p m", a=DT), wv[:])

                    # ======== phase B: QKV + RoPE (pre-normed x^T input) ====
                    with tc.tile_pool(name="phB", bufs=1) as pb, \
                         tc.tile_pool(name="psB", bufs=1, space="PSUM") as psB:

                        def rope(ps, dst, gc, tag):
                            csx = cs_cat[:, gc]
                            scx = sc_cat[:, gc]
                            a = pb.tile([64, 512], F32, name=f"ra_{tag}",
                                        tag="ra", bufs=2)
                            b_ = pb.tile([64, 512], F32, name=f"rb_{tag}",
                                         tag="rb", bufs=2)
                            nc.vector.tensor_mul(a[:], ps[0:64, :], csx[0:64, :])
                            nc.vector.tensor_mul(b_[:], ps[64:128, :],
                                                 csx[64:128, :])
                            nc.vector.tensor_sub(dst[0:64, gc], a[:], b_[:])
                            c_ = pb.tile([64, 512], F32, name=f"rc_{tag}",
                                         tag="rc", bufs=2)
                            d_ = pb.tile([64, 512], F32, name=f"rd_{tag}",
                                         tag="rd", bufs=2)
                            nc.vector.tensor_mul(c_[:], ps[0:64, :], scx[0:64, :])
                            nc.vector.tensor_mul(d_[:], ps[64:128, :],
                                                 scx[64:128, :])
                            nc.vector.tensor_add(dst[64:128, gc], c_[:], d_[:])

                        for g in range(NG):
                            gc = slice(g * 512, (g + 1) * 512)
                            xng = pb.tile([128, DT * 512], BF16, name=f"xng{g}",
                                          tag="xng", bufs=2)
                            nc.sync.dma_start(
                                xng[:].rearrange("p (a t) -> a p t", a=DT),
                                xnt[:, :, gc])

                            def xg(d):
                                return xng[:, d * 512:(d + 1) * 512]

                            for hl in range(2):
                                ps = psB.tile([128, 512], F32, name=f"psq{hl}_{g}",
                                              tag="pqk", bufs=2)
                                for k in range(DT):
                                    nc.tensor.matmul(
                                        ps[:],
                                        wq_sb[:, k * 256 + hl * DH:
                                              k * 256 + (hl + 1) * DH],
                                        xg(k), start=(k == 0), stop=(k == DT - 1))
                                rope(ps, qT[hl], gc, f"q{hl}_{g}")
                            ps = psB.tile([128, 512], F32, name=f"psk_{g}",
                                          tag="pqk", bufs=2)
                            for k in range(DT):
                                nc.tensor.matmul(
                                    ps[:], wk_sb[:, k * DH:(k + 1) * DH],
                                    xg(k), start=(k == 0), stop=(k == DT - 1))
                            rope(ps, kT, gc, f"k{g}")
                            # V directly in [token, dh] layout (flipped matmul)
                            for tt in range(4):
                                pv = psB.tile([128, DH], F32, name=f"pv{g}_{tt}",
                                              tag="pv", bufs=2)
                                for k in range(DT):
                                    nc.tensor.matmul(
                                        pv[:],
                                        xg(k)[:, tt * 128:(tt + 1) * 128],
                                        wv_sb[:, k * DH:(k + 1) * DH],
                                        start=(k == 0), stop=(k == DT - 1))
                                nc.scalar.copy(Vn[g * 4 + tt][:], pv[:])

                    # ======== phase C: attention ========
                    with tc.tile_pool(name="phC", bufs=1) as pc, \
                         tc.tile_pool(name="psC", bufs=1, space="PSUM") as psC:
                        for b2 in range(B):
                            for hl in range(2):
                                for qg in range(2):
                                    qc = slice(b2 * T + qg * 512,
                                               b2 * T + (qg + 1) * 512)
                                    nkt = 4 * (qg + 1)
                                    pso = psC.tile([128, 512], F32,
                                                   name=f"pso{b2}{hl}{qg}",
                                                   tag="pso", bufs=2)
                                    pssum = psC.tile([1, 512], F32,
                                                     name=f"pssum{b2}{hl}{qg}",
                                                     tag="pssum", bufs=2)
                                    for kt in range(nkt):
                                        pss = psC.tile([128, 512], F32,
                                                       name=f"pss{b2}{hl}{qg}{kt}",
                                                       tag="pss", bufs=3)
                                        k0 = b2 * T + kt * 128
                                        nc.tensor.matmul(
                                            pss[:], kT[:, k0:k0 + 128],
                                            qT[hl][:, qc], start=True, stop=True)
                                        e = pc.tile([128, 512], BF16,
                                                    name=f"e{b2}{hl}{qg}{kt}",
                                                    tag="e", bufs=4)
                                        nc.scalar.activation(e[:], pss[:], AF.Exp,
                                                             scale=scexp[:])
                                        v = kt - 4 * qg
                                        if 0 <= v <= 3:
                                            em = pc.tile([128, 512], BF16,
                                                         name=f"em{b2}{hl}{qg}{kt}",
                                                         tag="em", bufs=2)
                                            nc.vector.tensor_mul(
                                                em[:], e[:],
                                                msk[:, v * 512:(v + 1) * 512])
                                            eu = em
                                        else:
                                            eu = e
                                        nc.tensor.matmul(
                                            pssum[:], ones_cb[:], eu[:],
                                            start=(kt == 0), stop=(kt == nkt - 1))
                                        nc.tensor.matmul(
                                            pso[:], Vn[b2 * 8 + kt][:], eu[:],
                                            start=(kt == 0), stop=(kt == nkt - 1))
                                    rec = pc.tile([1, 512], F32R,
                                                  name=f"rec{b2}{hl}{qg}",
                                                  tag="rec", bufs=2)
                                    with nc.allow_low_precision(
                                            reason="f32r softmax recip"):
                                        nc.vector.reciprocal(rec[:], pssum[:])
                                    rbc = psC.tile([128, 512], F32,
                                                   name=f"rbc{b2}{hl}{qg}",
                                                   tag="rbc", bufs=1)
                                    nc.tensor.matmul(rbc[:], ones_r[:], rec[:],
                                                     start=True, stop=True)
                                    rbs = pc.tile([128, 512], F32,
                                                  name=f"rbs{b2}{hl}{qg}",
                                                  tag="rbs", bufs=2)
                                    nc.vector.tensor_copy(rbs[:], rbc[:])
                                    nc.vector.tensor_mul(oT[hl][:, qc],
                                                         pso[:], rbs[:])

                # A2A: my heads x all tokens -> all heads x my tokens
                for hl in range(2):
                    nc.sync.dma_start(
                        o_in[:, hl * 128:(hl + 1) * 128, :],
                        oT[hl][:].rearrange("p (j t) -> j p t", j=NC))
                nc.gpsimd.collective_compute(
                    "AllToAll", mybir.AluOpType.bypass, replica_groups=RG,
                    ins=[o_in[:]], outs=[o_out[:]])

                # ======== phase D: o-proj + residual + norm2 ========
                with tc.tile_pool(name="phD", bufs=1) as pd, \
                     tc.tile_pool(name="psD", bufs=1, space="PSUM") as psD:
                    oT_own = pd.tile([128, DT * TOK], BF16, name="oT_own")
                    oo = o_out.rearrange("j (a p) t -> (j a) p t", p=128)
                    nc.sync.dma_start(
                        oT_own[:].rearrange("p (k t) -> k p t", k=DT), oo)
                    for d in range(DT):
                        wos = pd.tile([128, D], BF16, name=f"wos{d}",
                                      tag="wos", bufs=4)
                        nc.scalar.dma_start(wos[:], wo[d])
                        pso2 = psD.tile([128, TOK], F32, name=f"pso2_{d}",
                                        tag="pso2", bufs=2)
                        for k in range(DT):
                            nc.tensor.matmul(
                                pso2[:], wos[:, k * 128:(k + 1) * 128],
                                oT_own[:, k * TOK:(k + 1) * TOK],
                                start=(k == 0), stop=(k == DT - 1))
                        nc.vector.tensor_add(
                            x1T[d][:], pso2[:],
                            xts[:, d * TOK:(d + 1) * TOK])
                    # norm2 (transposed): ssq over partitions via ones-matmul
                    ssq2 = psD.tile([1, TOK], F32, name="ssq2")
                    for d in range(DT):
                        sq2 = pd.tile([128, TOK], F32R, name=f"sq2_{d}",
                                      tag="sq2", bufs=2)
                        nc.scalar.activation(sq2[:], x1T[d][:], AF.Square)
                        nc.tensor.matmul(ssq2[:], ones_cr[:], sq2[:],
                                         start=(d == 0), stop=(d == DT - 1))
                    std2 = pd.tile([1, TOK], F32, name="std2")
                    nc.scalar.activation(std2[:], ssq2[:], AF.Sqrt,
                                         scale=scd1[:], bias=eps1[:])
                    inv2 = pd.tile([1, TOK], F32R, name="inv2")
                    with nc.allow_low_precision(reason="f32r norm2 recip"):
                        nc.vector.reciprocal(inv2[:], std2[:])
                    i2p = psD.tile([128, TOK], F32, name="i2p")
                    nc.tensor.matmul(i2p[:], ones_r[:], inv2[:],
                                     start=True, stop=True)
                    i2s = pd.tile([128, TOK], F32, name="i2s")
                    nc.vector.tensor_copy(i2s[:], i2p[:])
                    for d in range(DT):
                        nc.vector.tensor_mul(hT[d][:], x1T[d][:], i2s[:])

                # ======== phase E: FFN (token-sharded, streamed weights) ====
                with tc.tile_pool(name="phE", bufs=1) as pe, \
                     tc.tile_pool(name="psE", bufs=1, space="PSUM") as psE:
                    for j in range(NFP):
                        w1t = pe.tile([128, 4096], BF16, name=f"w1t{j}",
                                      tag="w1t", bufs=3)
                        nc.gpsimd.dma_start(w1t[:], w1p[j])
                        w3t = pe.tile([128, 4096], BF16, name=f"w3t{j}",
                                      tag="w3t", bufs=3)
                        nc.gpsimd.dma_start(w3t[:], w3p[j])
                        for s in range(2):
                            ft = 2 * j + s
                            pg = psE.tile([128, TOK], F32, name=f"pg{ft}",
                                          tag="pg", bufs=2)
                            for k in range(DT):
                                nc.tensor.matmul(
                                    pg[:],
                                    w1t[:, s * 2048 + k * 128:
                                        s * 2048 + (k + 1) * 128],
                                    hT[k][:], start=(k == 0), stop=(k == DT - 1))
                            pu = psE.tile([128, TOK], F32, name=f"pu{ft}",
                                          tag="pu", bufs=2)
                            for k in range(DT):
                                nc.tensor.matmul(
                                    pu[:],
                                    w3t[:, s * 2048 + k * 128:
                                        s * 2048 + (k + 1) * 128],
                                    hT[k][:], start=(k == 0), stop=(k == DT - 1))
                            sil = pe.tile([128, TOK], F32, name=f"sil{ft}",
                                          tag="sil", bufs=3)
                            nc.scalar.activation(sil[:], pg[:], AF.Silu)
                            nc.vector.tensor_mul(zT[ft][:], sil[:], pu[:])
                    for d in range(DT):
                        w2t = pe.tile([128, F], BF16, name=f"w2t{d}",
                                      tag="w2t", bufs=2)
                        nc.scalar.dma_start(w2t[:], w2d[d])
                        pf = psE.tile([128, TOK], F32, name=f"pf{d}",
                                      tag="pf", bufs=2)
                        for ft in range(NFT):
                            nc.tensor.matmul(
                                pf[:], w2t[:, ft * 128:(ft + 1) * 128],
                                zT[ft][:], start=(ft == 0), stop=(ft == NFT - 1))
                        fo = pe.tile([128, TOK], F32, name=f"fo{d}",
                                     tag="fo", bufs=3)
                        nc.vector.tensor_add(fo[:], pf[:], x1T[d][:])
                        nc.sync.dma_start(yt[d * 128:(d + 1) * 128, :], fo[:])
    nc.compile()
    return nc


def _prep_inputs(inputs):
    import ml_dtypes
    BF = ml_dtypes.bfloat16

    x = np.asarray(inputs["x"], np.float32)
    cos = np.asarray(inputs["freqs_cos"], np.float32)
    sin = np.asarray(inputs["freqs_sin"], np.float32)
    wn1 = np.asarray(inputs["w_norm1"], np.float32)[:, None]
    wn2 = np.asarray(inputs["w_norm2"], np.float32)[:, None]
    wq = np.asarray(inputs["wq"], np.float32) * wn1
    wk = np.asarray(inputs["wk"], np.float32) * wn1
    wv = np.asarray(inputs["wv"], np.float32) * wn1
    wo = np.asarray(inputs["wo"], np.float32)
    w1 = np.asarray(inputs["w1"], np.float32) * wn2
    w3 = np.asarray(inputs["w3"], np.float32) * wn2
    w2 = np.asarray(inputs["w2"], np.float32)

    xg = np.ascontiguousarray(x.reshape(GLOB, D))
    # host-side rmsnorm (norm1) + transpose
    inv1 = 1.0 / np.sqrt(np.mean(xg.astype(np.float64) ** 2, axis=1) + EPS)
    xn = (xg * inv1[:, None].astype(np.float32))
    xnt = np.ascontiguousarray(xn.T).reshape(DT, 128, GLOB).astype(BF)
    xgt = np.ascontiguousarray(xg.T)  # [D, GLOB] fp32

    perm = np.concatenate([np.arange(0, DH, 2), np.arange(1, DH, 2)])
    cosT = np.concatenate([cos.T, cos.T], axis=1)
    sinT = np.concatenate([sin.T, sin.T], axis=1)
    cscat = np.ascontiguousarray(np.concatenate([cosT, sinT], axis=0)).astype(BF)
    sccat = np.ascontiguousarray(np.concatenate([sinT, cosT], axis=0)).astype(BF)
    mk = np.zeros((4, 128, 512), np.float32)
    for v in range(4):
        r = np.arange(128)[:, None] + v * 128
        q = np.arange(512)[None, :]
        mk[v] = (r <= q).astype(np.float32)
    mk = mk.astype(BF)

    wo_sw = np.ascontiguousarray(
        wo.reshape(DT, 128, DT, 128).transpose(2, 1, 0, 3)
        .reshape(DT, 128, D)).astype(BF)
    # w1/w3 packed as ft-pairs: [22, 128, 2*2048], sub-block s then k-major
    w1pp = np.ascontiguousarray(
        w1.reshape(DT, 128, NFP, 2, 128).transpose(2, 1, 3, 0, 4)
        .reshape(NFP, 128, 4096)).astype(BF)
    w3pp = np.ascontiguousarray(
        w3.reshape(DT, 128, NFP, 2, 128).transpose(2, 1, 3, 0, 4)
        .reshape(NFP, 128, 4096)).astype(BF)
    # w2 packed d-major: [16, 128, 5632]: w2dd[d, p, ft*128+c] = w2[ft*128+p, d*128+c]
    w2dd = np.ascontiguousarray(
        w2.reshape(NFT, 128, DT, 128).transpose(2, 1, 0, 3)
        .reshape(DT, 128, F)).astype(BF)

    in_maps = []
    for c in range(NC):
        g = c // 2
        wq_c = np.empty((D, 2 * DH), np.float32)
        for hl in range(2):
            h = 2 * c + hl
            wq_c[:, hl * DH:(hl + 1) * DH] = wq[:, h * DH + perm]
        wk_c = wk[:, g * DH + perm]
        wv_c = wv[:, g * DH:(g + 1) * DH]
        in_maps.append({
            "xnt": xnt,
            "xtc": np.ascontiguousarray(
                xgt[:, c * TOK:(c + 1) * TOK]).reshape(DT, 128, TOK),
            "wq": np.ascontiguousarray(wq_c).reshape(DT, 128, 2 * DH).astype(BF),
            "wk": np.ascontiguousarray(wk_c).reshape(DT, 128, DH).astype(BF),
            "wv": np.ascontiguousarray(wv_c).reshape(DT, 128, DH).astype(BF),
            "wo": wo_sw,
            "w1p": w1pp,
            "w3p": w3pp,
            "w2d": w2dd,
            "cscat": cscat,
            "sccat": sccat,
            "masks": mk,
        })
    return in_maps


def kernel(**inputs) -> np.ndarray:
    from concourse import bass_utils

    if "nc" not in _CACHE:
        _CACHE["nc"] = _build_program()
    nc = _CACHE["nc"]
    in_maps = _prep_inputs(inputs)
    res = bass_utils.run_bass_kernel_spmd(nc, in_maps, core_ids=list(range(NC)))
    yT = np.empty((D, GLOB), np.float32)
    for c in range(NC):
        yT[:, c * TOK:(c + 1) * TOK] = res.results[c]["yt"]
    return np.ascontiguousarray(yT.T).reshape(B, T, D)


if __name__ == "__main__":
    import reference
    inputs = {k: np.asarray(v) for k, v in reference.setup_inputs().items()}
    out = kernel(**inputs)
    print("kernel output shape:", out.shape)
